# revision 2
# baseline (speedup 1.0000x reference)
"""Multi-head attention (B=4, L=2048, D=1024, H=16) on 8 trn2 NeuronCores.

Sharding: core c -> (batch b = c//2, head-group hg = c%2 of 8 heads).
Each core computes its batch's projections restricted to its 8 heads,
full attention for those (b, h) pairs, returning:
  attn [8, 2048, 2048] f32  and  out [2048, 512] f32
which the host reassembles into the full outputs.

Precision: fp32-grade via bf16 hi/lo "K-stacking" (one K=2048 matmul
computes the exact (hi+lo)x(hi+lo) product at 2x bf16 cost). The A@V pass
uses fp16 attention weights (relative error ~2.4e-4).

Softmax without any transposes of the big matrices:
  pass A ([k,q] layout): S.T -> exp -> E.T fp16 -> A@V with a ones-column
     appended to V, so PSUM row 64 accumulates the softmax denominators.
  tiny PE transposes of [65,128] chunks give O and the sums in [q,*] layout;
  DVE reciprocal -> per-partition scales.
  pass B ([q,k] layout): S -> exp -> multiply by 1/sum (per-partition
     scalar, DVE 2x mode) -> DMA straight to HBM in the natural layout.
"""
import numpy as np
import ml_dtypes
from contextlib import ExitStack

import concourse.bass as bass
import concourse.mybir as mybir
import concourse.tile as tile
from concourse.bass import ds
from concourse.bass_utils import run_bass_kernel_spmd
from concourse.masks import make_identity

AF = mybir.ActivationFunctionType
F32 = mybir.dt.float32
BF16 = mybir.dt.bfloat16
F16 = mybir.dt.float16

B, L, D, H = 4, 2048, 1024, 16
HG = 8          # heads per core
DH = 64         # head dim
P = 128
NCORES = 8
SCALE = 1.0 / 8.0   # 1/sqrt(DH)

_wait_ctr = [0]


def _split_sync_waits(nc, limit=1):
    """walrus (CoreV3) rejects instructions with too many sem waits; hoist
    excess waits onto preceding same-engine NoOps."""
    for f in nc.m.functions:
        for bb in f.blocks:
            out = []
            for inst in bb.instructions:
                si = getattr(inst, "sync_info", None)
                if si is not None and si.on_wait and len(si.on_wait) > limit:
                    waits = list(si.on_wait)
                    keep = waits[-limit:]
                    extra = waits[:-limit]
                    for i in range(0, len(extra), limit):
                        _wait_ctr[0] += 1
                        nop = mybir.InstNoOp(
                            name=f"waitsplit_nop_{_wait_ctr[0]}", ins=[], outs=[]
                        )
                        nop.engine = inst.engine
                        nop.sync_info = mybir.SyncInfo(
                            on_wait=extra[i : i + limit], on_update=[]
                        )
                        out.append(nop)
                    inst.sync_info = mybir.SyncInfo(
                        on_wait=keep, on_update=list(si.on_update)
                    )
                out.append(inst)
            bb.instructions = out
    return nc


def build_nc():
    nc = bass.Bass("TRN2", target_bir_lowering=False, debug=False)

    xs = {
        t: nc.dram_tensor(f"xs_{t}", [2 * D, L], BF16, kind="ExternalInput")
        for t in "qkv"
    }
    ws = {
        t: nc.dram_tensor(f"ws_{t}", [2 * D, HG * DH], BF16, kind="ExternalInput")
        for t in "qkv"
    }
    attn_o = nc.dram_tensor("attn", [HG, L, L], F32, kind="ExternalOutput")
    out_o = nc.dram_tensor("out", [L, HG * DH], F32, kind="ExternalOutput")

    NLB = L // 512       # 4 l-blocks
    NEO = (2 * D) // P   # 16 stacked-contraction chunks
    NKC = L // P         # 16 k-chunks
    NQC = L // P         # 16 q-chunks

    with tile.TileContext(nc) as tc, ExitStack() as ctx:
        pers = ctx.enter_context(tc.tile_pool(name="pers", bufs=1))
        qs_sb = pers.tile([P, HG, L], BF16, tag="qs")
        ks_sb = pers.tile([P, HG, L], BF16, tag="ks")
        v_sb = pers.tile([P, NKC, HG * 65], F16, tag="v")
        out_sb = pers.tile([P, NQC, HG * DH], F32, tag="osb")
        ident = pers.tile([P, P], F32, tag="ident")
        make_identity(nc, ident[:])
        # ones-columns of the augmented V (data columns overwritten below)
        nc.vector.memset(v_sb[:], 1.0)

        # ---------------- Phase 1: projections ----------------
        with ExitStack() as c1:
            wpool = c1.enter_context(tc.tile_pool(name="wsb", bufs=1))
            xpool = c1.enter_context(tc.tile_pool(name="xsb", bufs=2))
            hpool = c1.enter_context(tc.tile_pool(name="hilo", bufs=3))
            ppsum = c1.enter_context(tc.tile_pool(name="pps", bufs=2, space="PSUM"))

            ws_sb = {}
            for t in "qkv":
                ws_sb[t] = wpool.tile([P, NEO, HG * DH], BF16, tag=f"ws{t}", name=f"ws{t}")
                nc.sync.dma_start(
                    ws_sb[t][:], ws[t].ap().rearrange("(eo p) d -> p eo d", p=P)
                )

            for t in "qkv":
                dst = qs_sb if t == "q" else (ks_sb if t == "k" else None)
                for lb in range(NLB):
                    xt = xpool.tile([P, NEO, 512], BF16, tag="x")
                    nc.sync.dma_start(
                        xt[:],
                        xs[t].ap().rearrange("(eo p) l -> p eo l", p=P)[
                            :, :, ds(lb * 512, 512)
                        ],
                    )
                    if t in "qk":
                        # transposed layout: psum [128 dchunk, 512 l]
                        for dc in range(4):
                            ps = ppsum.tile([P, 512], F32, tag="ps")
                            for eo in range(NEO):
                                nc.tensor.matmul(
                                    ps[:],
                                    ws_sb[t][:, eo, ds(dc * P, P)],
                                    xt[:, eo, :],
                                    start=(eo == 0),
                                    stop=(eo == NEO - 1),
                                )
                            hi = hpool.tile([P, 512], BF16, tag="hi")
                            lo = hpool.tile([P, 512], BF16, tag="lo")
                            nc.vector.tensor_copy(hi[:], ps[:])
                            nc.vector.tensor_tensor(
                                lo[:], ps[:], hi[:], mybir.AluOpType.subtract
                            )
                            # assemble per-head [hi;lo] stacks (partition moves
                            # via SBUF->SBUF DMA)
                            for h2 in range(2):
                                h = 2 * dc + h2
                                nc.sync.dma_start(
                                    dst[0:64, h, ds(lb * 512, 512)],
                                    hi[ds(h2 * 64, 64), :],
                                )
                                nc.sync.dma_start(
                                    dst[64:128, h, ds(lb * 512, 512)],
                                    lo[ds(h2 * 64, 64), :],
                                )
                    else:
                        # natural layout: psum [128 l, 512 d]
                        for lc2 in range(4):
                            ps = ppsum.tile([P, 512], F32, tag="ps")
                            for eo in range(NEO):
                                nc.tensor.matmul(
                                    ps[:],
                                    xt[:, eo, ds(lc2 * P, P)],
                                    ws_sb["v"][:, eo, :],
                                    start=(eo == 0),
                                    stop=(eo == NEO - 1),
                                )
                            lc = lb * 4 + lc2
                            vv = v_sb[:, lc].rearrange("p (h x) -> p h x", h=HG)
                            nc.vector.tensor_copy(
                                vv[:, :, 0:64],
                                ps[:].rearrange("p (h d) -> p h d", h=HG),
                            )

        # ---------------- Phase 2: attention ----------------
        etp = ctx.enter_context(tc.tile_pool(name="et", bufs=3))
        e2p = ctx.enter_context(tc.tile_pool(name="e2", bufs=3))
        pstg = ctx.enter_context(tc.tile_pool(name="pst", bufs=2))
        oaugp = ctx.enter_context(tc.tile_pool(name="oaug", bufs=2))
        rp = ctx.enter_context(tc.tile_pool(name="recip", bufs=2))
        st_ps = ctx.enter_context(tc.tile_pool(name="stp", bufs=1, space="PSUM"))
        av_ps = ctx.enter_context(tc.tile_pool(name="avp", bufs=2, space="PSUM"))
        tr_ps = ctx.enter_context(tc.tile_pool(name="trp", bufs=2, space="PSUM"))
        sb_ps = ctx.enter_context(tc.tile_pool(name="sbp", bufs=1, space="PSUM"))

        for h in range(HG):
            rec = rp.tile([P, NQC], F32, tag="rec")
            for qbp in range(2):  # pairs of 512-q-blocks -> 1024 q at a time
                avs = [av_ps.tile([65, 512], F32, tag="av", name=f"av{i}") for i in range(2)]
                for kc in range(NKC):
                    st = st_ps.tile([P, 1024], F32, tag="st")
                    for qh in range(2):
                        nc.tensor.matmul(
                            st[:, ds(qh * 512, 512)],
                            ks_sb[:, h, ds(kc * P, P)],
                            qs_sb[:, h, ds(qbp * 1024 + qh * 512, 512)],
                            start=True,
                            stop=True,
                        )
                    et = etp.tile([P, 1024], F16, tag="et")
                    nc.scalar.activation(et[:], st[:], AF.Exp, scale=SCALE)
                    for qh in range(2):
                        nc.tensor.matmul(
                            avs[qh][:],
                            v_sb[:, kc, ds(h * 65, 65)],
                            et[:, ds(qh * 512, 512)],
                            start=(kc == 0),
                            stop=(kc == NKC - 1),
                        )
                oaug = oaugp.tile([65, 1024], F32, tag="oaug")
                nc.scalar.copy(oaug[:, 0:512], avs[0][:])
                nc.scalar.copy(oaug[:, 512:1024], avs[1][:])
                for half in range(2):
                    tr = tr_ps.tile([P, 4, 65], F32, tag="tr")
                    for j in range(4):
                        chunk = half * 4 + j
                        nc.tensor.transpose(
                            tr[:, j, :],
                            oaug[:, ds(chunk * P, P)],
                            ident[0:65, 0:65],
                        )
                    qc0 = qbp * 8 + half * 4
                    nc.vector.reciprocal(rec[:, ds(qc0, 4)], tr[:, :, 64])
                    for j in range(4):
                        qc = qc0 + j
                        nc.scalar.activation(
                            out_sb[:, qc, ds(h * DH, DH)],
                            tr[:, j, 0:DH],
                            AF.Copy,
                            scale=rec[:, ds(qc, 1)],
                        )
            # pass B: natural-layout S -> normalized attention weights
            for qc in range(NQC):
                pstage = pstg.tile([P, L], F32, tag="pstage")
                for kh in range(2):
                    sb = sb_ps.tile([P, 1024], F32, tag="sb")
                    for kb2 in range(2):
                        nc.tensor.matmul(
                            sb[:, ds(kb2 * 512, 512)],
                            qs_sb[:, h, ds(qc * P, P)],
                            ks_sb[:, h, ds(kh * 1024 + kb2 * 512, 512)],
                            start=True,
                            stop=True,
                        )
                    e2 = e2p.tile([P, 1024], F32, tag="e2")
                    nc.scalar.activation(e2[:], sb[:], AF.Exp, scale=SCALE)
                    nc.vector.tensor_scalar_mul(
                        pstage[:, ds(kh * 1024, 1024)], e2[:], rec[:, ds(qc, 1)]
                    )
                nc.sync.dma_start(attn_o.ap()[h, ds(qc * P, P), :], pstage[:])

        for qc in range(NQC):
            nc.sync.dma_start(out_o.ap()[ds(qc * P, P), :], out_sb[:, qc, :])

    return _split_sync_waits(nc)


_NC = None


def _get_nc():
    global _NC
    if _NC is None:
        _NC = build_nc()
    return _NC


def _prep_in_maps(query, key, value, w_q, w_k, w_v):
    query = np.asarray(query, dtype=np.float32)
    key = np.asarray(key, dtype=np.float32)
    value = np.asarray(value, dtype=np.float32)
    w = {
        "q": np.asarray(w_q, dtype=np.float32),
        "k": np.asarray(w_k, dtype=np.float32),
        "v": np.asarray(w_v, dtype=np.float32),
    }
    x = {"q": query, "k": key, "v": value}

    def hilo(a):  # [n, m] f32 -> [2n, m] bf16 stacked hi/lo
        hi = a.astype(ml_dtypes.bfloat16)
        lo = (a - hi.astype(np.float32)).astype(ml_dtypes.bfloat16)
        return np.ascontiguousarray(np.concatenate([hi, lo], axis=0))

    ws_c = {}  # per head-group weight stacks
    for t in "qkv":
        ws_c[t] = [
            hilo(np.ascontiguousarray(w[t][hg * HG * DH : (hg + 1) * HG * DH, :].T))
            for hg in range(2)
        ]
    in_maps = []
    for c in range(NCORES):
        b, hg = divmod(c, 2)
        m = {}
        for t in "qkv":
            m[f"xs_{t}"] = hilo(np.ascontiguousarray(x[t][b].T))
            m[f"ws_{t}"] = ws_c[t][hg]
        in_maps.append(m)
    return in_maps


def _assemble(results):
    out = np.empty((B, L, D), np.float32)
    attn = np.empty((B, H, L, L), np.float32)
    for c in range(NCORES):
        b, hg = divmod(c, 2)
        attn[b, hg * HG : (hg + 1) * HG] = results[c]["attn"]
        out[b, :, hg * HG * DH : (hg + 1) * HG * DH] = results[c]["out"]
    return out, attn


def run(in_maps, trace=False, **kwargs):
    nc = _get_nc()
    return run_bass_kernel_spmd(
        nc, in_maps, core_ids=list(range(NCORES)), trace=trace, **kwargs
    )


def kernel(query, key, value, w_q, w_k, w_v):
    in_maps = _prep_in_maps(query, key, value, w_q, w_k, w_v)
    res = run(in_maps)
    return _assemble(res.results)
